# revision 6
# baseline (speedup 1.0000x reference)
"""Trainium2 Bass kernel for nn_BiLinear (synthetic EMLP BiLinear).

Math: out[b,o] = 0.05 * sum_i x[b,i] * Wflat[b, perm[o*512+i]]
where Wflat[b,k] is a small GEMM of param rows against gathered x columns:
  k < M0:  Wflat[b,k] = sum_{n<128} p0[k,n] * x[b, bids0[n]]
  k >= M0: (m,s) = divmod(k-M0,12); Wflat[b,k] = sum_{n<32} p1[m,n] * x[b, bids1[n*12+s]]

Since perm is a permutation of the full 512x512 (o,i) grid, we host-reorder the
param rows into "grid order": pgrid[:, o*512+i] holds the param row of cell
(o,i), scattered into a unified 512-tall contraction space
  q in [0,128)          -> gathered col bids0[q]
  q = 128 + s*32 + n    -> gathered col bids1[n*12+s]
Then on device:
  xgall[q,b] = x[b, colmap[q]]                      (one-hot matmuls)
  V[b, o*512+i] = sum_q xgall[q,b] * pgrid[q, cell] (4 accumulating matmuls/row)
  out[b,o] = 0.05 * sum_i V[b,(o,i)] * x[b,i]       (fused tensor_tensor_reduce)
No gathers/scatters on device; all index work is host-side data prep.

Sharding: output rows o are split across the 8 cores (64 each). x and the
gather matrix are replicated; pgrid is partitioned. No collectives; the host
concatenates the per-core (256, 64) outputs.
"""

import os
import sys

import numpy as np

if "/opt/trn_rl_repo" not in sys.path:
    sys.path.insert(0, "/opt/trn_rl_repo")

# Problem constants (hardcoded per contract).
S0, S1 = 1, 12
N0, N1 = 128, 32
M0, M1 = 22144, 20000
DIN, DOUT = 512, 512
WSIZE = DOUT * DIN
B = 256
NCORES = 8
OSH = DOUT // NCORES  # output rows per core
KCH = 4  # contraction chunks of 128
CELLS = OSH * DIN  # grid cells per core

# "f32" or "bf16" for the big GEMM operands (pgrid + xgall stationary).
_DT_MODE = os.environ.get("KERNEL_DTYPE", "bf16")

_NC_CACHE = {}
LAST_EXEC_NS = None
LAST_RESULTS = None


def _np_dt(mode):
    import ml_dtypes

    return {
        "f32": np.float32,
        "f32r": np.float32,
        "bf16": ml_dtypes.bfloat16,
        "f16": np.float16,
    }[mode]


def _prep(w, bids0, bids1, matrix_perm, mode):
    """Host-side data prep: gather matrix + per-core grid-ordered param slabs."""
    w = np.asarray(w, np.float32)
    bids0 = np.asarray(bids0, np.int64)
    bids1 = np.asarray(bids1, np.int64)
    mp = np.asarray(matrix_perm, np.int64)
    p0 = w[: M0 * N0].reshape(M0, N0)
    p1 = w[M0 * N0 :].reshape(M1, N1)

    colmap = np.empty(512, np.int64)
    colmap[:128] = bids0
    s_idx = np.arange(S1)
    n_idx = np.arange(N1)
    # q = 128 + s*32 + n  ->  bids1[n*12 + s]
    colmap[128:] = bids1[(n_idx[None, :] * S1 + s_idx[:, None])].reshape(384)
    gmat = np.zeros((512, 512), np.float32)
    gmat[colmap, np.arange(512)] = 1.0

    dt = _np_dt(mode)
    nrow = np.arange(N1)
    slabs = []
    for c in range(NCORES):
        k = mp[c * CELLS : (c + 1) * CELLS]
        pg = np.zeros((512, CELLS), np.float32)
        j0 = np.nonzero(k < M0)[0]
        pg[:128, j0] = p0[k[j0]].T
        j1 = np.nonzero(k >= M0)[0]
        m1, s1 = np.divmod(k[j1] - M0, S1)
        rows = 128 + s1 * N1
        pg[(rows[:, None] + nrow[None, :]), j1[:, None]] = p1[m1]
        # (512, OSH*512) -> (OSH, 128p, KCH*512): per-o-row tile contiguous in DRAM
        pg = pg.reshape(KCH, 128, OSH, DIN).transpose(2, 1, 0, 3)
        slabs.append(np.ascontiguousarray(pg.reshape(OSH, 128, KCH * DIN)).astype(dt))
    return gmat, slabs


def _build_nc(mode):
    import concourse.bacc as bacc
    import concourse.tile as tile
    from concourse import mybir
    from concourse.masks import make_identity

    f32 = mybir.dt.float32
    dt_mm = {
        "f32": f32,
        "f32r": mybir.dt.float32r,
        "bf16": mybir.dt.bfloat16,
        "f16": mybir.dt.float16,
    }[mode]

    nc = bacc.Bacc("TRN2", target_bir_lowering=False, debug=False, num_devices=NCORES)
    x_d = nc.dram_tensor("x", (B, DIN), f32, kind="ExternalInput").ap()
    g_d = nc.dram_tensor("gmat", (DIN, DIN), f32, kind="ExternalInput").ap()
    pg_d = nc.dram_tensor("pg", (OSH, 128, KCH * DIN), dt_mm, kind="ExternalInput").ap()
    out_d = nc.dram_tensor("out", (B, OSH), f32, kind="ExternalOutput").ap()

    with tile.TileContext(nc) as tc:
        with (
            tc.tile_pool(name="const", bufs=1) as cp,
            tc.tile_pool(name="pgp", bufs=3) as pgp,
            tc.tile_pool(name="zp", bufs=3) as zp,
            tc.tile_pool(name="psv", bufs=4, space="PSUM") as psv,
            tc.tile_pool(name="pst", bufs=2, space="PSUM") as pst,
        ):
            ident = cp.tile([128, 128], f32, name="ident")
            make_identity(nc, ident)

            x_sb = []
            for h in range(2):
                t = cp.tile([128, DIN], f32, name=f"x{h}")
                nc.sync.dma_start(t[:], x_d[h * 128 : (h + 1) * 128, :])
                x_sb.append(t)
            g_sb = []
            for c in range(4):
                t = cp.tile([128, DIN], f32, name=f"g{c}")
                nc.sync.dma_start(t[:], g_d[c * 128 : (c + 1) * 128, :])
                g_sb.append(t)

            # xT[c] = x[:, 128c:128c+128].T  -> (128 i, 256 b)
            xT = []
            for c in range(4):
                t = cp.tile([128, B], f32, name=f"xT{c}")
                for h in range(2):
                    ps = pst.tile([128, 128], f32, name="tp")
                    nc.tensor.transpose(
                        ps[:], x_sb[h][:, c * 128 : (c + 1) * 128], ident[:]
                    )
                    nc.any.tensor_copy(out=t[:, h * 128 : (h + 1) * 128], in_=ps[:])
                xT.append(t)

            # xgT[g][q - 128g, b] = x[b, colmap[q]] via one-hot matmuls
            xgT = []
            for g in range(4):
                ps = pst.tile([128, B], f32, name="xg")
                for c in range(4):
                    nc.tensor.matmul(
                        ps[:],
                        lhsT=g_sb[c][:, g * 128 : (g + 1) * 128],
                        rhs=xT[c][:],
                        start=(c == 0),
                        stop=(c == 3),
                    )
                t = cp.tile([128, B], dt_mm, name=f"xgT{g}")
                nc.any.tensor_copy(out=t[:], in_=ps[:])
                xgT.append(t)

            oacc = [cp.tile([128, OSH], f32, name=f"oacc{h}") for h in range(2)]

            for o in range(OSH):
                pg_t = pgp.tile([128, KCH, DIN], dt_mm, name="pgt")
                nc.sync.dma_start(
                    pg_t[:], pg_d[o].rearrange("p (c n) -> p c n", c=KCH)
                )
                for h in range(2):
                    v = psv.tile([128, DIN], f32, name="v")
                    for c in range(KCH):
                        nc.tensor.matmul(
                            v[:],
                            lhsT=xgT[c][:, h * 128 : (h + 1) * 128],
                            rhs=pg_t[:, c, :],
                            start=(c == 0),
                            stop=(c == KCH - 1),
                        )
                    z = zp.tile([128, DIN], f32, name="z")
                    nc.vector.tensor_mul(out=z[:], in0=v[:], in1=x_sb[h][:])
                    z2 = zp.tile([128, DIN], f32, name="z2")
                    nc.scalar.activation(
                        out=z2[:],
                        in_=z[:],
                        func=mybir.ActivationFunctionType.Copy,
                        scale=0.05,
                        accum_out=oacc[h][:, o : o + 1],
                    )

            for h in range(2):
                nc.sync.dma_start(out_d[h * 128 : (h + 1) * 128, :], oacc[h][:])

    nc.compile()
    return nc


def kernel(x, w, bids0, bids1, matrix_perm):
    global LAST_EXEC_NS, LAST_RESULTS
    from concourse import bass_utils

    mode = _DT_MODE
    x = np.ascontiguousarray(np.asarray(x, np.float32))
    gmat, slabs = _prep(w, bids0, bids1, matrix_perm, mode)

    if mode not in _NC_CACHE:
        _NC_CACHE[mode] = _build_nc(mode)
    nc = _NC_CACHE[mode]

    in_maps = [{"x": x, "gmat": gmat, "pg": slabs[c]} for c in range(NCORES)]
    try:
        res = bass_utils.run_bass_kernel_spmd(nc, in_maps, core_ids=list(range(NCORES)))
    except ModuleNotFoundError:
        # Tracing (BASS_TRACE=1) requires the axon NTFF hook; fall back to no-trace.
        os.environ["BASS_NEVER_TRACE"] = "1"
        res = bass_utils.run_bass_kernel_spmd(nc, in_maps, core_ids=list(range(NCORES)))
    LAST_RESULTS = res
    LAST_EXEC_NS = res.exec_time_ns

    out = np.empty((B, DOUT), np.float32)
    for c in range(NCORES):
        out[:, c * OSH : (c + 1) * OSH] = res.results[c]["out"]
    return out


# revision 8
# speedup vs baseline: 1.1428x; 1.1428x over previous
"""Trainium2 Bass kernel for nn_BiLinear (synthetic EMLP BiLinear).

Math: out[b,o] = 0.05 * sum_i x[b,i] * Wflat[b, perm[o*512+i]]
where Wflat[b,k] is a small GEMM of param rows against gathered x columns:
  k < M0:  Wflat[b,k] = sum_{n<128} p0[k,n] * x[b, bids0[n]]
  k >= M0: (m,s) = divmod(k-M0,12); Wflat[b,k] = sum_{n<32} p1[m,n] * x[b, bids1[n*12+s]]

Since perm is a permutation of the full 512x512 (o,i) grid, we host-reorder the
param rows into "grid order": pgrid[:, o*512+i] holds the param row of cell
(o,i), scattered into a unified 512-tall contraction space
  q in [0,128)          -> gathered col bids0[q]
  q = 128 + s*32 + n    -> gathered col bids1[n*12+s]
Then on device:
  xgall[q,b] = x[b, colmap[q]]                      (one-hot matmuls)
  V[b, o*512+i] = sum_q xgall[q,b] * pgrid[q, cell] (4 accumulating matmuls/row)
  out[b,o] = 0.05 * sum_i V[b,(o,i)] * x[b,i]       (fused tensor_tensor_reduce)
No gathers/scatters on device; all index work is host-side data prep.

Sharding: output rows o are split across the 8 cores (64 each). x and the
gather matrix are replicated; pgrid is partitioned. No collectives; the host
concatenates the per-core (256, 64) outputs.
"""

import os
import sys

import numpy as np

if "/opt/trn_rl_repo" not in sys.path:
    sys.path.insert(0, "/opt/trn_rl_repo")

# Problem constants (hardcoded per contract).
S0, S1 = 1, 12
N0, N1 = 128, 32
M0, M1 = 22144, 20000
DIN, DOUT = 512, 512
WSIZE = DOUT * DIN
B = 256
NCORES = 8
OSH = DOUT // NCORES  # output rows per core
KCH = 4  # contraction chunks of 128
CELLS = OSH * DIN  # grid cells per core

# "f32" or "bf16" for the big GEMM operands (pgrid + xgall stationary).
_DT_MODE = os.environ.get("KERNEL_DTYPE", "bf16")

_NC_CACHE = {}
LAST_EXEC_NS = None
LAST_RESULTS = None


def _np_dt(mode):
    import ml_dtypes

    return {
        "f32": np.float32,
        "f32r": np.float32,
        "bf16": ml_dtypes.bfloat16,
        "f16": np.float16,
    }[mode]


def _prep(w, bids0, bids1, matrix_perm, mode):
    """Host-side data prep: gather matrix + per-core grid-ordered param slabs."""
    w = np.asarray(w, np.float32)
    bids0 = np.asarray(bids0, np.int64)
    bids1 = np.asarray(bids1, np.int64)
    mp = np.asarray(matrix_perm, np.int64)
    p0 = w[: M0 * N0].reshape(M0, N0)
    p1 = w[M0 * N0 :].reshape(M1, N1)

    colmap = np.empty(512, np.int64)
    colmap[:128] = bids0
    s_idx = np.arange(S1)
    n_idx = np.arange(N1)
    # q = 128 + s*32 + n  ->  bids1[n*12 + s]
    colmap[128:] = bids1[(n_idx[None, :] * S1 + s_idx[:, None])].reshape(384)
    gmat = np.zeros((512, 512), np.float32)
    gmat[colmap, np.arange(512)] = 1.0

    dt = _np_dt(mode)
    nrow = np.arange(N1)
    slabs = []
    for c in range(NCORES):
        k = mp[c * CELLS : (c + 1) * CELLS]
        pg = np.zeros((512, CELLS), np.float32)
        j0 = np.nonzero(k < M0)[0]
        pg[:128, j0] = p0[k[j0]].T
        j1 = np.nonzero(k >= M0)[0]
        m1, s1 = np.divmod(k[j1] - M0, S1)
        rows = 128 + s1 * N1
        pg[(rows[:, None] + nrow[None, :]), j1[:, None]] = p1[m1]
        # (512, OSH*512) -> (OSH, 128p, KCH*512): per-o-row tile contiguous in DRAM
        pg = pg.reshape(KCH, 128, OSH, DIN).transpose(2, 1, 0, 3)
        slabs.append(np.ascontiguousarray(pg.reshape(OSH, 128, KCH * DIN)).astype(dt))
    return gmat, slabs


def _build_nc(mode):
    import concourse.bacc as bacc
    import concourse.tile as tile
    from concourse import mybir
    from concourse.masks import make_identity

    f32 = mybir.dt.float32
    dt_mm = {
        "f32": f32,
        "f32r": mybir.dt.float32r,
        "bf16": mybir.dt.bfloat16,
        "f16": mybir.dt.float16,
    }[mode]

    nc = bacc.Bacc("TRN2", target_bir_lowering=False, debug=False, num_devices=NCORES)
    x_d = nc.dram_tensor("x", (B, DIN), f32, kind="ExternalInput").ap()
    g_d = nc.dram_tensor("gmat", (DIN, DIN), f32, kind="ExternalInput").ap()
    pg_d = nc.dram_tensor("pg", (OSH, 128, KCH * DIN), dt_mm, kind="ExternalInput").ap()
    out_d = nc.dram_tensor("out", (B, OSH), f32, kind="ExternalOutput").ap()

    with tile.TileContext(nc) as tc:
        with (
            tc.tile_pool(name="const", bufs=1) as cp,
            tc.tile_pool(name="pgp", bufs=4) as pgp,
            tc.tile_pool(name="zp", bufs=4) as zp,
            tc.tile_pool(name="psv", bufs=6, space="PSUM") as psv,
            tc.tile_pool(name="pst", bufs=1, space="PSUM") as pst,
        ):
            ident = cp.tile([128, 128], f32, name="ident")
            make_identity(nc, ident)

            x_sb = []
            for h in range(2):
                t = cp.tile([128, DIN], f32, name=f"x{h}")
                nc.sync.dma_start(t[:], x_d[h * 128 : (h + 1) * 128, :])
                x_sb.append(t)
            g_sb = []
            for c in range(4):
                t = cp.tile([128, DIN], f32, name=f"g{c}")
                nc.sync.dma_start(t[:], g_d[c * 128 : (c + 1) * 128, :])
                g_sb.append(t)

            # xT[c] = x[:, 128c:128c+128].T  -> (128 i, 256 b)
            xT = []
            for c in range(4):
                t = cp.tile([128, B], f32, name=f"xT{c}")
                for h in range(2):
                    ps = pst.tile([128, 128], f32, name="tp")
                    nc.tensor.transpose(
                        ps[:], x_sb[h][:, c * 128 : (c + 1) * 128], ident[:]
                    )
                    nc.any.tensor_copy(out=t[:, h * 128 : (h + 1) * 128], in_=ps[:])
                xT.append(t)

            # xgT[g][q - 128g, b] = x[b, colmap[q]] via one-hot matmuls
            xgT = []
            for g in range(4):
                ps = pst.tile([128, B], f32, name="xg")
                for c in range(4):
                    nc.tensor.matmul(
                        ps[:],
                        lhsT=g_sb[c][:, g * 128 : (g + 1) * 128],
                        rhs=xT[c][:],
                        start=(c == 0),
                        stop=(c == 3),
                    )
                t = cp.tile([128, B], dt_mm, name=f"xgT{g}")
                nc.any.tensor_copy(out=t[:], in_=ps[:])
                xgT.append(t)

            oacc = [cp.tile([128, OSH], f32, name=f"oacc{h}") for h in range(2)]

            for o in range(OSH):
                pg_t = pgp.tile([128, KCH, DIN], dt_mm, name="pgt")
                nc.sync.dma_start(
                    pg_t[:], pg_d[o].rearrange("p (c n) -> p c n", c=KCH)
                )
                for h in range(2):
                    v = psv.tile([128, DIN], f32, name="v")
                    for c in range(KCH):
                        nc.tensor.matmul(
                            v[:],
                            lhsT=xgT[c][:, h * 128 : (h + 1) * 128],
                            rhs=pg_t[:, c, :],
                            start=(c == 0),
                            stop=(c == KCH - 1),
                        )
                    z = zp.tile([128, DIN], mybir.dt.bfloat16, name="z")
                    nc.vector.tensor_mul(out=z[:], in0=v[:], in1=x_sb[h][:])
                    z2 = zp.tile([128, DIN], mybir.dt.bfloat16, name="z2")
                    nc.scalar.activation(
                        out=z2[:],
                        in_=z[:],
                        func=mybir.ActivationFunctionType.Copy,
                        scale=0.05,
                        accum_out=oacc[h][:, o : o + 1],
                    )

            for h in range(2):
                nc.sync.dma_start(out_d[h * 128 : (h + 1) * 128, :], oacc[h][:])

    nc.compile()
    return nc


def kernel(x, w, bids0, bids1, matrix_perm):
    global LAST_EXEC_NS, LAST_RESULTS
    from concourse import bass_utils

    mode = _DT_MODE
    x = np.ascontiguousarray(np.asarray(x, np.float32))
    gmat, slabs = _prep(w, bids0, bids1, matrix_perm, mode)

    if mode not in _NC_CACHE:
        _NC_CACHE[mode] = _build_nc(mode)
    nc = _NC_CACHE[mode]

    in_maps = [{"x": x, "gmat": gmat, "pg": slabs[c]} for c in range(NCORES)]
    try:
        res = bass_utils.run_bass_kernel_spmd(nc, in_maps, core_ids=list(range(NCORES)))
    except ModuleNotFoundError:
        # Tracing (BASS_TRACE=1) requires the axon NTFF hook; fall back to no-trace.
        os.environ["BASS_NEVER_TRACE"] = "1"
        res = bass_utils.run_bass_kernel_spmd(nc, in_maps, core_ids=list(range(NCORES)))
    LAST_RESULTS = res
    LAST_EXEC_NS = res.exec_time_ns

    out = np.empty((B, DOUT), np.float32)
    for c in range(NCORES):
        out[:, c * OSH : (c + 1) * OSH] = res.results[c]["out"]
    return out


# revision 10
# speedup vs baseline: 1.2628x; 1.1050x over previous
"""Trainium2 Bass kernel for nn_BiLinear (synthetic EMLP BiLinear).

Math: out[b,o] = 0.05 * sum_i x[b,i] * Wflat[b, perm[o*512+i]]
where Wflat[b,k] is a small GEMM of param rows against gathered x columns:
  k < M0:  Wflat[b,k] = sum_{n<128} p0[k,n] * x[b, bids0[n]]
  k >= M0: (m,s) = divmod(k-M0,12); Wflat[b,k] = sum_{n<32} p1[m,n] * x[b, bids1[n*12+s]]

Since perm is a permutation of the full 512x512 (o,i) grid, we host-reorder the
param rows into "grid order": pgrid[:, o*512+i] holds the param row of cell
(o,i), scattered into a unified 512-tall contraction space
  q in [0,128)          -> gathered col bids0[q]
  q = 128 + s*32 + n    -> gathered col bids1[n*12+s]
Then on device:
  xgall[q,b] = x[b, colmap[q]]                      (one-hot matmuls)
  V[b, o*512+i] = sum_q xgall[q,b] * pgrid[q, cell] (4 accumulating matmuls/row)
  out[b,o] = 0.05 * sum_i V[b,(o,i)] * x[b,i]       (fused tensor_tensor_reduce)
No gathers/scatters on device; all index work is host-side data prep.

Sharding: output rows o are split across the 8 cores (64 each). x and the
gather matrix are replicated; pgrid is partitioned. No collectives; the host
concatenates the per-core (256, 64) outputs.
"""

import os
import sys

import numpy as np

if "/opt/trn_rl_repo" not in sys.path:
    sys.path.insert(0, "/opt/trn_rl_repo")

# Problem constants (hardcoded per contract).
S0, S1 = 1, 12
N0, N1 = 128, 32
M0, M1 = 22144, 20000
DIN, DOUT = 512, 512
WSIZE = DOUT * DIN
B = 256
NCORES = 8
OSH = DOUT // NCORES  # output rows per core
KCH = 4  # contraction chunks of 128
CELLS = OSH * DIN  # grid cells per core

# "f32" or "bf16" for the big GEMM operands (pgrid + xgall stationary).
_DT_MODE = os.environ.get("KERNEL_DTYPE", "bf16")

_NC_CACHE = {}
LAST_EXEC_NS = None
LAST_RESULTS = None


def _np_dt(mode):
    import ml_dtypes

    return {
        "f32": np.float32,
        "f32r": np.float32,
        "bf16": ml_dtypes.bfloat16,
        "f16": np.float16,
    }[mode]


def _prep(w, bids0, bids1, matrix_perm, mode):
    """Host-side data prep: gather matrix + per-core grid-ordered param slabs."""
    w = np.asarray(w, np.float32)
    bids0 = np.asarray(bids0, np.int64)
    bids1 = np.asarray(bids1, np.int64)
    mp = np.asarray(matrix_perm, np.int64)
    p0 = w[: M0 * N0].reshape(M0, N0)
    p1 = w[M0 * N0 :].reshape(M1, N1)

    colmap = np.empty(512, np.int64)
    colmap[:128] = bids0
    s_idx = np.arange(S1)
    n_idx = np.arange(N1)
    # q = 128 + s*32 + n  ->  bids1[n*12 + s]
    colmap[128:] = bids1[(n_idx[None, :] * S1 + s_idx[:, None])].reshape(384)
    gmat = np.zeros((512, 512), np.float32)
    gmat[colmap, np.arange(512)] = 1.0

    dt = _np_dt(mode)
    nrow = np.arange(N1)
    slabs = []
    for c in range(NCORES):
        k = mp[c * CELLS : (c + 1) * CELLS]
        pg = np.zeros((512, CELLS), np.float32)
        j0 = np.nonzero(k < M0)[0]
        pg[:128, j0] = p0[k[j0]].T
        j1 = np.nonzero(k >= M0)[0]
        m1, s1 = np.divmod(k[j1] - M0, S1)
        rows = 128 + s1 * N1
        pg[(rows[:, None] + nrow[None, :]), j1[:, None]] = p1[m1]
        # (512, OSH*512) -> (OSH, 128p, KCH*512): per-o-row tile contiguous in DRAM
        pg = pg.reshape(KCH, 128, OSH, DIN).transpose(2, 1, 0, 3)
        slabs.append(np.ascontiguousarray(pg.reshape(OSH, 128, KCH * DIN)).astype(dt))
    return gmat, slabs


def _build_nc(mode):
    import concourse.bacc as bacc
    import concourse.tile as tile
    from concourse import mybir
    from concourse.masks import make_identity

    f32 = mybir.dt.float32
    dt_mm = {
        "f32": f32,
        "f32r": mybir.dt.float32r,
        "bf16": mybir.dt.bfloat16,
        "f16": mybir.dt.float16,
    }[mode]

    nc = bacc.Bacc("TRN2", target_bir_lowering=False, debug=False, num_devices=NCORES)
    x_d = nc.dram_tensor("x", (B, DIN), f32, kind="ExternalInput").ap()
    g_d = nc.dram_tensor("gmat", (DIN, DIN), f32, kind="ExternalInput").ap()
    pg_d = nc.dram_tensor("pg", (OSH, 128, KCH * DIN), dt_mm, kind="ExternalInput").ap()
    out_d = nc.dram_tensor("out", (B, OSH), f32, kind="ExternalOutput").ap()

    with tile.TileContext(nc) as tc:
        with (
            tc.tile_pool(name="const", bufs=1) as cp,
            tc.tile_pool(name="pgp", bufs=4) as pgp,
            tc.tile_pool(name="zp", bufs=4) as zp,
            tc.tile_pool(name="psv", bufs=6, space="PSUM") as psv,
            tc.tile_pool(name="pst", bufs=1, space="PSUM") as pst,
        ):
            ident = cp.tile([128, 128], f32, name="ident")
            make_identity(nc, ident)

            # Warm the PE clock (HAM) during the prologue with dummy matmuls.
            warm = psv.tile([128, DIN], f32, name="warm", tag="v")
            for _ in range(32):
                nc.tensor.matmul(
                    warm[:, :128], lhsT=ident[:], rhs=ident[:], start=True, stop=True
                )

            x_sb = []
            for h in range(2):
                t = cp.tile([128, DIN], f32, name=f"x{h}")
                nc.sync.dma_start(t[:], x_d[h * 128 : (h + 1) * 128, :])
                x_sb.append(t)
            g_sb = []
            for c in range(4):
                t = cp.tile([128, DIN], f32, name=f"g{c}")
                nc.sync.dma_start(t[:], g_d[c * 128 : (c + 1) * 128, :])
                g_sb.append(t)

            # xT[c] = x[:, 128c:128c+128].T  -> (128 i, 256 b)
            xT = []
            for c in range(4):
                t = cp.tile([128, B], f32, name=f"xT{c}")
                for h in range(2):
                    ps = pst.tile([128, 128], f32, name="tp")
                    nc.tensor.transpose(
                        ps[:], x_sb[h][:, c * 128 : (c + 1) * 128], ident[:]
                    )
                    nc.any.tensor_copy(out=t[:, h * 128 : (h + 1) * 128], in_=ps[:])
                xT.append(t)

            # xgT[g][q - 128g, b] = x[b, colmap[q]] via one-hot matmuls
            xgT = []
            for g in range(4):
                ps = pst.tile([128, B], f32, name="xg")
                for c in range(4):
                    nc.tensor.matmul(
                        ps[:],
                        lhsT=g_sb[c][:, g * 128 : (g + 1) * 128],
                        rhs=xT[c][:],
                        start=(c == 0),
                        stop=(c == 3),
                    )
                t = cp.tile([128, B], dt_mm, name=f"xgT{g}")
                nc.any.tensor_copy(out=t[:], in_=ps[:])
                xgT.append(t)

            oacc = [cp.tile([128, OSH], f32, name=f"oacc{h}") for h in range(2)]

            for o in range(OSH):
                pg_t = pgp.tile([128, KCH, DIN], dt_mm, name="pgt")
                nc.sync.dma_start(
                    pg_t[:], pg_d[o].rearrange("p (c n) -> p c n", c=KCH)
                )
                for h in range(2):
                    v = psv.tile([128, DIN], f32, name="v", tag="v")
                    for c in range(KCH):
                        nc.tensor.matmul(
                            v[:],
                            lhsT=xgT[c][:, h * 128 : (h + 1) * 128],
                            rhs=pg_t[:, c, :],
                            start=(c == 0),
                            stop=(c == KCH - 1),
                        )
                    z = zp.tile([128, DIN], mybir.dt.bfloat16, name="z")
                    if (2 * o + h) % 9 < 2:
                        # fused mul+reduce on VectorE (balances ScalarE load)
                        nc.vector.scalar_tensor_tensor(
                            out=z[:],
                            in0=v[:],
                            scalar=0.05,
                            in1=x_sb[h][:],
                            op0=mybir.AluOpType.mult,
                            op1=mybir.AluOpType.mult,
                            accum_out=oacc[h][:, o : o + 1],
                        )
                    else:
                        nc.vector.tensor_mul(out=z[:], in0=v[:], in1=x_sb[h][:])
                        z2 = zp.tile([128, DIN], mybir.dt.bfloat16, name="z2")
                        nc.scalar.activation(
                            out=z2[:],
                            in_=z[:],
                            func=mybir.ActivationFunctionType.Copy,
                            scale=0.05,
                            accum_out=oacc[h][:, o : o + 1],
                        )

            for h in range(2):
                nc.sync.dma_start(out_d[h * 128 : (h + 1) * 128, :], oacc[h][:])

    nc.compile()
    return nc


def kernel(x, w, bids0, bids1, matrix_perm):
    global LAST_EXEC_NS, LAST_RESULTS
    from concourse import bass_utils

    mode = _DT_MODE
    x = np.ascontiguousarray(np.asarray(x, np.float32))
    gmat, slabs = _prep(w, bids0, bids1, matrix_perm, mode)

    if mode not in _NC_CACHE:
        _NC_CACHE[mode] = _build_nc(mode)
    nc = _NC_CACHE[mode]

    in_maps = [{"x": x, "gmat": gmat, "pg": slabs[c]} for c in range(NCORES)]
    try:
        res = bass_utils.run_bass_kernel_spmd(nc, in_maps, core_ids=list(range(NCORES)))
    except ModuleNotFoundError:
        # Tracing (BASS_TRACE=1) requires the axon NTFF hook; fall back to no-trace.
        os.environ["BASS_NEVER_TRACE"] = "1"
        res = bass_utils.run_bass_kernel_spmd(nc, in_maps, core_ids=list(range(NCORES)))
    LAST_RESULTS = res
    LAST_EXEC_NS = res.exec_time_ns

    out = np.empty((B, DOUT), np.float32)
    for c in range(NCORES):
        out[:, c * OSH : (c + 1) * OSH] = res.results[c]["out"]
    return out


# revision 12
# speedup vs baseline: 1.3027x; 1.0316x over previous
"""Trainium2 Bass kernel for nn_BiLinear (synthetic EMLP BiLinear).

Math: out[b,o] = 0.05 * sum_i x[b,i] * Wflat[b, perm[o*512+i]]
where Wflat[b,k] is a small GEMM of param rows against gathered x columns:
  k < M0:  Wflat[b,k] = sum_{n<128} p0[k,n] * x[b, bids0[n]]
  k >= M0: (m,s) = divmod(k-M0,12); Wflat[b,k] = sum_{n<32} p1[m,n] * x[b, bids1[n*12+s]]

Since perm is a permutation of the full 512x512 (o,i) grid, we host-reorder the
param rows into "grid order": pgrid[:, o*512+i] holds the param row of cell
(o,i), scattered into a unified 512-tall contraction space
  q in [0,128)          -> gathered col bids0[q]
  q = 128 + s*32 + n    -> gathered col bids1[n*12+s]
Then on device:
  xgall[q,b] = x[b, colmap[q]]                      (one-hot matmuls)
  V[b, o*512+i] = sum_q xgall[q,b] * pgrid[q, cell] (4 accumulating matmuls/row)
  out[b,o] = 0.05 * sum_i V[b,(o,i)] * x[b,i]       (fused tensor_tensor_reduce)
No gathers/scatters on device; all index work is host-side data prep.

Sharding: output rows o are split across the 8 cores (64 each). x and the
gather matrix are replicated; pgrid is partitioned. No collectives; the host
concatenates the per-core (256, 64) outputs.
"""

import os
import sys

import numpy as np

if "/opt/trn_rl_repo" not in sys.path:
    sys.path.insert(0, "/opt/trn_rl_repo")

# Problem constants (hardcoded per contract).
S0, S1 = 1, 12
N0, N1 = 128, 32
M0, M1 = 22144, 20000
DIN, DOUT = 512, 512
WSIZE = DOUT * DIN
B = 256
NCORES = 8
OSH = DOUT // NCORES  # output rows per core
KCH = 4  # contraction chunks of 128
CELLS = OSH * DIN  # grid cells per core

# "f32" or "bf16" for the big GEMM operands (pgrid + xgall stationary).
_DT_MODE = os.environ.get("KERNEL_DTYPE", "bf16")

_NC_CACHE = {}
LAST_EXEC_NS = None
LAST_RESULTS = None


def _np_dt(mode):
    import ml_dtypes

    return {
        "f32": np.float32,
        "f32r": np.float32,
        "bf16": ml_dtypes.bfloat16,
        "f16": np.float16,
    }[mode]


def _prep(w, bids0, bids1, matrix_perm, mode):
    """Host-side data prep: gather matrix + per-core grid-ordered param slabs."""
    w = np.asarray(w, np.float32)
    bids0 = np.asarray(bids0, np.int64)
    bids1 = np.asarray(bids1, np.int64)
    mp = np.asarray(matrix_perm, np.int64)
    p0 = w[: M0 * N0].reshape(M0, N0)
    p1 = w[M0 * N0 :].reshape(M1, N1)

    colmap = np.empty(512, np.int64)
    colmap[:128] = bids0
    s_idx = np.arange(S1)
    n_idx = np.arange(N1)
    # q = 128 + s*32 + n  ->  bids1[n*12 + s]
    colmap[128:] = bids1[(n_idx[None, :] * S1 + s_idx[:, None])].reshape(384)
    gmat = np.zeros((512, 512), np.float32)
    gmat[colmap, np.arange(512)] = 1.0

    dt = _np_dt(mode)
    nrow = np.arange(N1)
    slabs = []
    for c in range(NCORES):
        k = mp[c * CELLS : (c + 1) * CELLS]
        pg = np.zeros((512, CELLS), np.float32)
        j0 = np.nonzero(k < M0)[0]
        pg[:128, j0] = p0[k[j0]].T
        j1 = np.nonzero(k >= M0)[0]
        m1, s1 = np.divmod(k[j1] - M0, S1)
        rows = 128 + s1 * N1
        pg[(rows[:, None] + nrow[None, :]), j1[:, None]] = p1[m1]
        # (512, OSH*512) -> (OSH, 128p, KCH*512): per-o-row tile contiguous in DRAM
        pg = pg.reshape(KCH, 128, OSH, DIN).transpose(2, 1, 0, 3)
        slabs.append(np.ascontiguousarray(pg.reshape(OSH, 128, KCH * DIN)).astype(dt))
    return gmat, slabs


def _build_nc(mode):
    import concourse.bacc as bacc
    import concourse.tile as tile
    from concourse import mybir
    from concourse.masks import make_identity

    f32 = mybir.dt.float32
    dt_mm = {
        "f32": f32,
        "f32r": mybir.dt.float32r,
        "bf16": mybir.dt.bfloat16,
        "f16": mybir.dt.float16,
    }[mode]

    nc = bacc.Bacc("TRN2", target_bir_lowering=False, debug=False, num_devices=NCORES)
    x_d = nc.dram_tensor("x", (B, DIN), f32, kind="ExternalInput").ap()
    g_d = nc.dram_tensor("gmat", (DIN, DIN), f32, kind="ExternalInput").ap()
    pg_d = nc.dram_tensor("pg", (OSH, 128, KCH * DIN), dt_mm, kind="ExternalInput").ap()
    out_d = nc.dram_tensor("out", (B, OSH), f32, kind="ExternalOutput").ap()

    with tile.TileContext(nc) as tc:
        with (
            tc.tile_pool(name="const", bufs=1) as cp,
            tc.tile_pool(name="pgp", bufs=6) as pgp,
            tc.tile_pool(name="zp", bufs=4) as zp,
            tc.tile_pool(name="psv", bufs=6, space="PSUM") as psv,
            tc.tile_pool(name="pst", bufs=1, space="PSUM") as pst,
        ):
            ident = cp.tile([128, 128], f32, name="ident")
            make_identity(nc, ident)

            x_sb = []
            for h in range(2):
                t = cp.tile([128, DIN], f32, name=f"x{h}")
                nc.sync.dma_start(t[:], x_d[h * 128 : (h + 1) * 128, :])
                x_sb.append(t)
            g_sb = []
            for c in range(4):
                t = cp.tile([128, DIN], f32, name=f"g{c}")
                nc.sync.dma_start(t[:], g_d[c * 128 : (c + 1) * 128, :])
                g_sb.append(t)

            # xT[c] = x[:, 128c:128c+128].T  -> (128 i, 256 b)
            xT = []
            for c in range(4):
                t = cp.tile([128, B], f32, name=f"xT{c}")
                for h in range(2):
                    ps = pst.tile([128, 128], f32, name="tp")
                    nc.tensor.transpose(
                        ps[:], x_sb[h][:, c * 128 : (c + 1) * 128], ident[:]
                    )
                    nc.any.tensor_copy(out=t[:, h * 128 : (h + 1) * 128], in_=ps[:])
                xT.append(t)

            # xgT[g][q - 128g, b] = x[b, colmap[q]] via one-hot matmuls
            xgT = []
            for g in range(4):
                ps = pst.tile([128, B], f32, name="xg")
                for c in range(4):
                    nc.tensor.matmul(
                        ps[:],
                        lhsT=g_sb[c][:, g * 128 : (g + 1) * 128],
                        rhs=xT[c][:],
                        start=(c == 0),
                        stop=(c == 3),
                    )
                t = cp.tile([128, B], dt_mm, name=f"xgT{g}")
                nc.any.tensor_copy(out=t[:], in_=ps[:])
                xgT.append(t)

            oacc = [cp.tile([128, OSH], f32, name=f"oacc{h}") for h in range(2)]

            for o in range(OSH):
                pg_t = pgp.tile([128, KCH, DIN], dt_mm, name="pgt")
                nc.sync.dma_start(
                    pg_t[:], pg_d[o].rearrange("p (c n) -> p c n", c=KCH)
                )
                for h in range(2):
                    v = psv.tile([128, DIN], f32, name="v", tag="v")
                    for c in range(KCH):
                        nc.tensor.matmul(
                            v[:],
                            lhsT=xgT[c][:, h * 128 : (h + 1) * 128],
                            rhs=pg_t[:, c, :],
                            start=(c == 0),
                            stop=(c == KCH - 1),
                        )
                    z = zp.tile([128, DIN], mybir.dt.bfloat16, name="z")
                    if (2 * o + h) % 9 < 2:
                        # fused mul+reduce on VectorE (balances ScalarE load)
                        nc.vector.scalar_tensor_tensor(
                            out=z[:],
                            in0=v[:],
                            scalar=0.05,
                            in1=x_sb[h][:],
                            op0=mybir.AluOpType.mult,
                            op1=mybir.AluOpType.mult,
                            accum_out=oacc[h][:, o : o + 1],
                        )
                    else:
                        nc.vector.tensor_mul(out=z[:], in0=v[:], in1=x_sb[h][:])
                        z2 = zp.tile([128, DIN], mybir.dt.bfloat16, name="z2")
                        nc.scalar.activation(
                            out=z2[:],
                            in_=z[:],
                            func=mybir.ActivationFunctionType.Copy,
                            scale=0.05,
                            accum_out=oacc[h][:, o : o + 1],
                        )

            for h in range(2):
                nc.sync.dma_start(out_d[h * 128 : (h + 1) * 128, :], oacc[h][:])

    nc.compile()
    return nc


def kernel(x, w, bids0, bids1, matrix_perm):
    global LAST_EXEC_NS, LAST_RESULTS
    from concourse import bass_utils

    mode = _DT_MODE
    x = np.ascontiguousarray(np.asarray(x, np.float32))
    gmat, slabs = _prep(w, bids0, bids1, matrix_perm, mode)

    if mode not in _NC_CACHE:
        _NC_CACHE[mode] = _build_nc(mode)
    nc = _NC_CACHE[mode]

    in_maps = [{"x": x, "gmat": gmat, "pg": slabs[c]} for c in range(NCORES)]
    try:
        res = bass_utils.run_bass_kernel_spmd(nc, in_maps, core_ids=list(range(NCORES)))
    except ModuleNotFoundError:
        # Tracing (BASS_TRACE=1) requires the axon NTFF hook; fall back to no-trace.
        os.environ["BASS_NEVER_TRACE"] = "1"
        res = bass_utils.run_bass_kernel_spmd(nc, in_maps, core_ids=list(range(NCORES)))
    LAST_RESULTS = res
    LAST_EXEC_NS = res.exec_time_ns

    out = np.empty((B, DOUT), np.float32)
    for c in range(NCORES):
        out[:, c * OSH : (c + 1) * OSH] = res.results[c]["out"]
    return out
